# revision 1
# baseline (speedup 1.0000x reference)
"""TRN2 Bass kernel for nn_Construct_76484777607483.

Computes, for 12 input tensors x_i [B=2, C=256, H=64, W=256]:
    y_i = einsum('bchw,co->bohw', x_i, W)
interleaved over H (output row 12*h + i comes from tensor i, row h) into
out [2, 256, 768, 256], plus bias b[o] * count(row) where count is the
conv-transpose overlap multiplicity (ramp 1..12 at the top edge, 12 in the
middle, 12..1 at the bottom edge).

Sharding: 8 cores = (2 batches) x (4 h-quarters of 16 input rows). Each core
handles all 12 tensors for its 16 rows, so the row-interleave is assembled
on-chip and output DMA writes are fully contiguous per channel.

Per-core kernel: for each group of 2 input rows (512 pixels), for each tensor
i, a [256 -> 256] channel matmul is done as 2 accumulating 128x128x512
matmuls in float32r (full-rate PE path, ~1.5e-4 rel err), then the PSUM tile
is copied into an interleave-layout SBUF buffer with the per-(i, h) bias
value added as a per-partition scalar (DVE tensor_scalar_add). The bias
values (b[o] * count) are precomputed on host per core.
"""

import numpy as np

import concourse.bacc as bacc
import concourse.tile as tile
import concourse.mybir as mybir
from concourse.bass_utils import run_bass_kernel_spmd

B, C, H, WD = 2, 256, 64, 256
NT = 12                 # stacked tensors
NCORES = 8
HQ = H // 4             # 16 input rows per core
NG = HQ // 2            # 8 groups of 2 rows
HOUT = NT * H           # 768

_F32 = mybir.dt.float32
_F32R = mybir.dt.float32r

_NC_CACHE = {}


def build_nc():
    if "nc" in _NC_CACHE:
        return _NC_CACHE["nc"]
    nc = bacc.Bacc("TRN2", target_bir_lowering=False)
    x_d = nc.declare_dram_parameter("x", [NT, C, HQ, WD], _F32R, isOutput=False)
    w_d = nc.declare_dram_parameter("w", [C, C], _F32R, isOutput=False)
    bv_d = nc.declare_dram_parameter("bv", [2, 128, NT * HQ], _F32, isOutput=False)
    y_d = nc.declare_dram_parameter("y", [C, NT * HQ, WD], _F32, isOutput=True)

    with tile.TileContext(nc) as tc:
        with (
            tc.tile_pool(name="const", bufs=1) as cpool,
            tc.tile_pool(name="xin", bufs=4) as inpool,
            tc.tile_pool(name="obuf", bufs=2) as outpool,
            tc.tile_pool(name="ps", bufs=4, space="PSUM") as pspool,
        ):
            wt = [
                [
                    cpool.tile([128, 128], _F32R, name=f"w{kh}{mh}")
                    for mh in range(2)
                ]
                for kh in range(2)
            ]
            for kh in range(2):
                for mh in range(2):
                    nc.sync.dma_start(
                        out=wt[kh][mh][:],
                        in_=w_d[kh * 128 : (kh + 1) * 128, mh * 128 : (mh + 1) * 128],
                    )
            bvt = [cpool.tile([128, NT * HQ], _F32, name=f"bv{mh}") for mh in range(2)]
            for mh in range(2):
                nc.sync.dma_start(out=bvt[mh][:], in_=bv_d[mh])

            for g in range(NG):
                obufs = [
                    outpool.tile(
                        [128, 2 * NT, WD], _F32, name=f"ob{g}_{mh}", tag=f"ob{mh}"
                    )
                    for mh in range(2)
                ]
                for i in range(NT):
                    xin = inpool.tile(
                        [128, 2, 2, WD], _F32R, name=f"xin{g}_{i}", tag="xin"
                    )
                    for kh in range(2):
                        nc.sync.dma_start(
                            out=xin[:, kh],
                            in_=x_d[i, kh * 128 : (kh + 1) * 128, 2 * g : 2 * g + 2, :],
                        )
                    for mh in range(2):
                        ps = pspool.tile(
                            [128, 2, WD], _F32, name=f"ps{g}_{i}_{mh}", tag="ps"
                        )
                        nc.tensor.matmul(
                            ps[:], wt[0][mh][:], xin[:, 0], start=True, stop=False
                        )
                        nc.tensor.matmul(
                            ps[:], wt[1][mh][:], xin[:, 1], start=False, stop=True
                        )
                        for hl in range(2):
                            col = i * HQ + 2 * g + hl
                            nc.vector.tensor_scalar_add(
                                obufs[mh][:, hl * NT + i],
                                ps[:, hl],
                                bvt[mh][:, col : col + 1],
                            )
                for mh in range(2):
                    nc.sync.dma_start(
                        out=y_d[
                            mh * 128 : (mh + 1) * 128, 24 * g : 24 * (g + 1), :
                        ],
                        in_=obufs[mh][:],
                    )
    nc.finalize()
    _NC_CACHE["nc"] = nc
    return nc


def _counts() -> np.ndarray:
    """count[r] for output row r (conv-transpose bias multiplicity)."""
    r = np.arange(HOUT)
    return (np.minimum(11, r) - np.maximum(0, r - (HOUT - NT)) + 1).astype(np.float32)


def shard_inputs(inputs: dict) -> list[dict]:
    xs = [np.ascontiguousarray(np.asarray(inputs[f"x{i}"], dtype=np.float32)) for i in range(NT)]
    w = np.ascontiguousarray(np.asarray(inputs["W"], dtype=np.float32))
    b = np.asarray(inputs["b"], dtype=np.float32)
    counts = _counts()
    in_maps = []
    for cid in range(NCORES):
        b_idx, hq = divmod(cid, 4)
        h0 = hq * HQ
        x_core = np.empty((NT, C, HQ, WD), dtype=np.float32)
        for i in range(NT):
            x_core[i] = xs[i][b_idx, :, h0 : h0 + HQ, :]
        # bv[mh, o, i*HQ + hl] = b[mh*128+o] * count(12*(h0+hl) + i)
        i_idx = np.arange(NT)[:, None]
        hl_idx = np.arange(HQ)[None, :]
        cnt = counts[12 * (h0 + hl_idx) + i_idx].reshape(NT * HQ)  # [192]
        bv = (b.reshape(2, 128)[:, :, None] * cnt[None, None, :]).astype(np.float32)
        in_maps.append({"x": x_core, "w": w, "bv": bv})
    return in_maps


def gather_outputs(results: list[dict]) -> np.ndarray:
    out = np.empty((B, C, HOUT, WD), dtype=np.float32)
    for cid in range(NCORES):
        b_idx, hq = divmod(cid, 4)
        h0 = hq * HQ
        out[b_idx, :, 12 * h0 : 12 * h0 + NT * HQ, :] = results[cid]["y"]
    return out


def kernel(**inputs) -> np.ndarray:
    nc = build_nc()
    in_maps = shard_inputs(inputs)
    res = run_bass_kernel_spmd(nc, in_maps, core_ids=list(range(NCORES)))
    return gather_outputs(res.results)
